# revision 1
# baseline (speedup 1.0000x reference)
"""CAM-module (channel attention) Trainium2 kernel.

Problem: B=4 samples, C=64, H=W=256 (N=65536 px). concat(rgb,hsv,lab) ->
X [192, N] per sample; q/k/v = 1x1-conv projections (W [64,192] + bias);
energy = q @ k^T * C^-0.5 -> softmax over last dim -> out = att @ v.

Sharding: 8 cores = 4 samples x 2 spatial halves (32768 px each). Each
core computes a partial energy over its half; a 16 KiB pairwise AllReduce
([[0,1],[2,3],[4,5],[6,7]]) completes the C x C energy, then each core
computes out for its own half.

Structure: X is cast to fp16 on the HOST (12.6 MB/core input DMA) and
kept RESIDENT in SBUF. Main loop computes only q/k (1-pass fp16, W hi
only) and the fp32 energy. v is never materialized: after softmax,
M^T = Wv_aug @ att^T is computed on-chip ([193,64], 2 tiny matmuls via a
host-uploaded Wv^T), and out = M X streams straight from the resident X
tiles. Output DMAs back as fp16; host casts to fp32.

Bias folds exactly via a ones-row baked into the host-side x1 upload and
bias rows on the weight chunks.
"""

import sys
import numpy as np

if '/opt/trn_rl_repo' not in sys.path:
    sys.path.insert(0, '/opt/trn_rl_repo')

B, C, H, W = 4, 64, 256, 256
N = H * W                 # 65536 px per sample
NHALF = N // 2            # 32768 px per core
PX = 2048                 # streaming tile (px)
NIT = NHALF // PX         # 16
SUB = 128                 # qkT subtile (px) = matmul M
NSUB = PX // SUB          # 16
VC = 512                  # out chunk (px) = matmul N
NVC = PX // VC            # 4
NCORES = 8

_CACHE = {}


def _build_bass(single_core=False):
    import concourse.bacc as bacc
    import concourse.mybir as mybir
    from concourse import tile

    F32 = mybir.dt.float32
    F16 = mybir.dt.float16
    Exp = mybir.ActivationFunctionType.Exp

    nc = bacc.Bacc("TRN2", target_bir_lowering=False, debug=False,
                   enable_asserts=False,
                   num_devices=1 if single_core else NCORES)

    x0_d = nc.dram_tensor("x0", [128, NHALF], F16, kind="ExternalInput").ap()
    x1_d = nc.dram_tensor("x1", [65, NHALF], F16, kind="ExternalInput").ap()
    # packed weights: cols [wqkh 0:128 | wv 128:192]
    w0_d = nc.dram_tensor("w0", [128, 192], F16, kind="ExternalInput").ap()
    w1_d = nc.dram_tensor("w1", [65, 192], F16, kind="ExternalInput").ap()
    wvT_d = nc.dram_tensor("wvT", [64, 193], F16, kind="ExternalInput").ap()
    ident_d = nc.dram_tensor("ident", [64, 64], F32, kind="ExternalInput").ap()
    out_d = nc.dram_tensor("out", [64, NHALF], F16, kind="ExternalOutput").ap()

    with tile.TileContext(nc) as tc:
        with tc.tile_pool(name="const", bufs=1) as const, \
             tc.tile_pool(name="qk", bufs=5) as qkpool, \
             tc.tile_pool(name="outp", bufs=4) as outp, \
             tc.tile_pool(name="qkps", bufs=3, space="PSUM") as qkps, \
             tc.tile_pool(name="vps", bufs=4, space="PSUM") as vps, \
             tc.tile_pool(name="eps", bufs=1, space="PSUM") as eps, \
             tc.tile_pool(name="dram", bufs=1, space="DRAM") as dram:

            # X stays resident in SBUF for the whole kernel — one tile pair
            # per 2048-px iteration so DMA prefetch overlaps compute. The
            # first pair's DMA is issued before everything else.
            x0t, x1t = [], []
            for it in range(NIT):
                x0c = const.tile([128, PX], F16, tag=f"x0_{it}")
                x0t.append(x0c)
                x1c = const.tile([65, PX], F16, tag=f"x1_{it}")
                x1t.append(x1c)
            nc.sync.dma_start(x0t[0][:], x0_d[:, 0:PX])
            nc.sync.dma_start(x1t[0][:], x1_d[:, 0:PX])

            w0 = const.tile([128, 192], F16)
            w1 = const.tile([65, 192], F16)
            wvT = const.tile([64, 193], F16)
            ident = const.tile([64, 64], F32)
            nc.sync.dma_start(w0[:], w0_d[:])
            nc.scalar.dma_start(w1[:], w1_d[:])
            nc.gpsimd.dma_start(wvT[:], wvT_d[:])
            nc.gpsimd.dma_start(ident[:], ident_d[:])
            wqkh0 = w0[:, 0:128]
            wqkh1 = w1[:, 0:128]

            # preload the ACT Exp table set off the critical path (~2.7us)
            warm = const.tile([1, 1], F32)
            nc.gpsimd.memset(warm[:], 0.0)
            nc.scalar.activation(warm[:], warm[:], Exp)

            ep = eps.tile([64, 64], F32)          # energy accumulator

            for it in range(NIT):
                sl = slice(it * PX, (it + 1) * PX)
                if it > 0:
                    nc.sync.dma_start(x0t[it][:], x0_d[:, sl])
                    nc.sync.dma_start(x1t[it][:], x1_d[:, sl])

                for grp in range(NSUB // 4):   # qkT: 4 subtiles per PSUM bank
                    qkp = qkps.tile([128, 512], F32, tag="qkp")
                    for s4 in range(4):
                        ssl = slice((grp * 4 + s4) * SUB, (grp * 4 + s4 + 1) * SUB)
                        osl = slice(s4 * 128, (s4 + 1) * 128)
                        nc.tensor.matmul(qkp[:, osl], x0t[it][:, ssl], wqkh0[:], start=True, stop=False)
                        nc.tensor.matmul(qkp[:, osl], x1t[it][:, ssl], wqkh1[:], start=False, stop=True)
                    qk_sb = qkpool.tile([128, 512], F32, tag="qk_sb")
                    if grp % 2 == 0:
                        nc.scalar.copy(qk_sb[:], qkp[:])
                    else:
                        nc.vector.tensor_copy(qk_sb[:], qkp[:])
                    for s4 in range(4):
                        first = (it == 0 and grp == 0 and s4 == 0)
                        last = (it == NIT - 1 and grp == NSUB // 4 - 1 and s4 == 3)
                        nc.tensor.matmul(ep[:], qk_sb[:, s4 * 128:s4 * 128 + 64],
                                         qk_sb[:, s4 * 128 + 64:s4 * 128 + 128],
                                         start=first, stop=last)

            # partial energy -> pairwise AllReduce
            e_sb = const.tile([64, 64], F32)
            nc.scalar.copy(e_sb[:], ep[:])
            bi = dram.tile([64, 64], F32)
            bo = dram.tile([64, 64], F32)
            nc.sync.dma_start(bi[:], e_sb[:])
            if single_core:
                nc.gpsimd.dma_start(bo[:], bi[:])
            else:
                nc.gpsimd.collective_compute(
                    "AllReduce", mybir.AluOpType.add,
                    replica_groups=[[0, 1], [2, 3], [4, 5], [6, 7]],
                    ins=[bi.opt()], outs=[bo.opt()],
                )
            e2 = const.tile([64, 64], F32)
            nc.sync.dma_start(e2[:], bo[:])

            # softmax over free dim, scale C^-0.5 = 0.125 folded into exp
            m = const.tile([64, 1], F32)
            nc.vector.reduce_max(m[:], e2[:], axis=mybir.AxisListType.X)
            mb = const.tile([64, 1], F32)
            nc.vector.tensor_scalar_mul(mb[:], m[:], -0.125)
            attu = const.tile([64, 64], F32)
            s = const.tile([64, 1], F32)
            nc.scalar.activation(attu[:], e2[:], Exp, bias=mb[:], scale=0.125,
                                 accum_out=s[:])
            r = const.tile([64, 1], F32)
            nc.vector.reciprocal(r[:], s[:])

            # attu^T (PE transpose, pre-normalization — the 1/s row scaling
            # folds into the out-stage copies), cast fp16
            atp = vps.tile([64, 64], F32, tag="vp")
            nc.tensor.transpose(atp[:], attu[:], ident[:])
            attT = const.tile([64, 64], F16)
            nc.scalar.copy(attT[:], atp[:])

            # M^T = Wv_aug @ attu^T  [193, 64]: fold att into the v-projection
            mt0p = vps.tile([128, 64], F32, tag="vp")
            nc.tensor.matmul(mt0p[:], wvT[:, 0:128], attT[:], start=True, stop=True)
            mt1p = vps.tile([65, 64], F32, tag="vp")
            nc.tensor.matmul(mt1p[:], wvT[:, 128:193], attT[:], start=True, stop=True)
            mt0 = const.tile([128, 64], F16)
            nc.scalar.copy(mt0[:], mt0p[:])
            mt1 = const.tile([65, 64], F16)
            nc.vector.tensor_copy(mt1[:], mt1p[:])

            # out = diag(1/s) M X straight from the resident X tiles,
            # [64,512] PSUM chunks 4-deep; MT0/MT1 each stay loaded across 4
            # matmuls, scaled copies alternate ACT/DVE, DMA per 2048px
            Ident = mybir.ActivationFunctionType.Identity
            for g in range(NHALF // PX):
                out_sb = outp.tile([64, PX], F16, tag="out_sb")
                ops = []
                for c in range(NVC):
                    op = vps.tile([64, VC], F32, tag="vp")
                    ops.append(op)
                for c in range(NVC):
                    csl = slice(c * VC, (c + 1) * VC)
                    nc.tensor.matmul(ops[c][:], mt0[:], x0t[g][:, csl],
                                     start=True, stop=False)
                for c in range(NVC):
                    csl = slice(c * VC, (c + 1) * VC)
                    nc.tensor.matmul(ops[c][:], mt1[:], x1t[g][:, csl],
                                     start=False, stop=True)
                for c in range(NVC):
                    osl = slice(c * VC, (c + 1) * VC)
                    if c % 2 == 0:
                        nc.scalar.activation(out_sb[:, osl], ops[c][:], Ident,
                                             scale=r[:])
                    else:
                        nc.vector.tensor_scalar_mul(out_sb[:, osl], ops[c][:], r[:])
                nc.sync.dma_start(out_d[:, g * PX:(g + 1) * PX], out_sb[:])

    nc.compile()
    return nc


def _get_nc():
    if 'nc' not in _CACHE:
        _CACHE['nc'] = _build_bass()
    return _CACHE['nc']


def _get_runner():
    """Build (once) a jitted shard_map executable for the bass module —
    the same executable shape run_bass_kernel_spmd uses under axon, but
    cached so repeat kernel() calls reuse it instead of tracing a second
    collective executable (which this relay's mesh can reject)."""
    if 'runner' in _CACHE:
        return _CACHE['runner']
    import jax
    from jax.sharding import Mesh, PartitionSpec, NamedSharding
    from jax.experimental.shard_map import shard_map
    import concourse.mybir as mybir
    from concourse import bass2jax

    nc = _get_nc()
    bass2jax.install_neuronx_cc_hook()
    in_names, out_names, out_avals = [], [], []
    for alloc in nc.m.functions[0].allocations:
        if not isinstance(alloc, mybir.MemoryLocationSet):
            continue
        name = alloc.memorylocations[0].name
        if alloc.kind == "ExternalInput":
            if (nc.partition_id_tensor is not None
                    and name == nc.partition_id_tensor.name):
                continue
            in_names.append(name)
        elif alloc.kind == "ExternalOutput":
            out_names.append(name)
            out_avals.append(jax.core.ShapedArray(tuple(alloc.tensor_shape),
                                                  mybir.dt.np(alloc.dtype)))
    n_params, n_outs = len(in_names), len(out_names)
    pid = nc.partition_id_tensor.name if nc.partition_id_tensor is not None else None

    def _body(*args):
        operands = list(args)
        names = in_names + out_names
        if pid is not None:
            operands.append(bass2jax.partition_id_tensor())
            names = names + [pid]
        return tuple(bass2jax._bass_exec_p.bind(
            *operands, out_avals=tuple(out_avals), in_names=tuple(names),
            out_names=tuple(out_names), lowering_input_output_aliases=(),
            sim_require_finite=False, sim_require_nnan=False, nc=nc))

    mesh = Mesh(np.asarray(jax.devices()[:NCORES]), ("core",))
    spec = PartitionSpec("core")
    fn = jax.jit(
        shard_map(_body, mesh=mesh, in_specs=(spec,) * (n_params + n_outs),
                  out_specs=(spec,) * n_outs, check_rep=False),
        donate_argnums=tuple(range(n_params, n_params + n_outs)),
        keep_unused=True)
    sh = NamedSharding(mesh, spec)
    _CACHE['runner'] = (fn, in_names, out_names, out_avals, sh)
    return _CACHE['runner']


def _run(in_maps):
    import jax
    fn, in_names, out_names, out_avals, sh = _get_runner()
    concat_in = [np.concatenate([np.asarray(in_maps[c][nm])
                                 for c in range(NCORES)], axis=0)
                 for nm in in_names]
    zeros = [np.zeros((NCORES * a.shape[0], *a.shape[1:]), a.dtype)
             for a in out_avals]
    dev_in = [jax.device_put(a, sh) for a in concat_in]
    outs = fn(*dev_in, *[jax.device_put(z, sh) for z in zeros])
    jax.block_until_ready(outs)
    return [
        {name: np.asarray(outs[i]).reshape(NCORES, *out_avals[i].shape)[c]
         for i, name in enumerate(out_names)}
        for c in range(NCORES)
    ]


def kernel(rgb, hsv, lab, Wq, bq, Wk, bk, Wv, bv):
    rgb = np.asarray(rgb, dtype=np.float32)
    hsv = np.asarray(hsv, dtype=np.float32)
    lab = np.asarray(lab, dtype=np.float32)
    Wq = np.asarray(Wq, dtype=np.float32)
    Wk = np.asarray(Wk, dtype=np.float32)
    Wv = np.asarray(Wv, dtype=np.float32)
    bq = np.asarray(bq, dtype=np.float32)
    bk = np.asarray(bk, dtype=np.float32)
    bv = np.asarray(bv, dtype=np.float32)

    # weight prep: [192ch + ones-row, outs] with bias row
    wqk = np.concatenate([Wq.T, Wk.T], axis=1)          # [192, 128]
    bqk = np.concatenate([bq, bk])                      # [128]
    wqk_aug = np.vstack([wqk, bqk[None, :]]).astype(np.float16)  # [193, 128]
    wv_aug = np.vstack([Wv.T, bv[None, :]]).astype(np.float16)   # [193, 64]

    shared = {
        "w0": np.ascontiguousarray(
            np.concatenate([wqk_aug[0:128], wv_aug[0:128]], axis=1)),
        "w1": np.ascontiguousarray(
            np.concatenate([wqk_aug[128:193], wv_aug[128:193]], axis=1)),
        "wvT": np.ascontiguousarray(wv_aug.T),
        "ident": np.eye(64, dtype=np.float32),
    }

    in_maps = []
    for c in range(NCORES):
        b, half = c // 2, c % 2
        hs = slice(half * (H // 2), (half + 1) * (H // 2))
        x0 = np.empty((128, NHALF), dtype=np.float16)
        x0[0:64] = rgb[b, :, hs, :].reshape(C, NHALF)
        x0[64:128] = hsv[b, :, hs, :].reshape(C, NHALF)
        x1 = np.empty((65, NHALF), dtype=np.float16)
        x1[0:64] = lab[b, :, hs, :].reshape(C, NHALF)
        x1[64] = 1.0
        in_maps.append({"x0": x0, "x1": x1, **shared})

    results = _run(in_maps)
    _CACHE['last_results'] = results
    _CACHE['last_in_maps'] = in_maps

    out = np.empty((B, C, H, W), dtype=np.float32)
    for c in range(NCORES):
        b, half = c // 2, c % 2
        hs = slice(half * (H // 2), (half + 1) * (H // 2))
        out[b, :, hs, :] = results[c]["out"].astype(np.float32).reshape(C, H // 2, W)
    return out

